# revision 1
# baseline (speedup 1.0000x reference)
"""Multi-head graph attention (rank-2 LeakyReLU-softmax) Trainium2 kernel.

Reference computation (per batch b, head h):
    V = X @ vW + vb                       (N, F)
    q = V @ qW[:,h] + qb[h]               (N,)   per-node scalar
    k = V @ kW[:,h] + kb[h]               (N,)
    A_ij = softmax_j( LeakyReLU(q_i * k_j) )
    out[b,i,h,:] = sum_j A_ij V_j

Key identity used here: with P = max(q,0), M = min(q,0),
alpha = LeakyReLU(k) = max(k, 0.01k), beta = min(k, 0.01k),
    LeakyReLU(q_i * k_j) == alpha_j * P_i + beta_j * M_i      (exactly)
since for each i exactly one of P_i / M_i is nonzero.  So the N x N logit
matrix is a rank-2 outer product, built on the TensorEngine as a K=2
matmul (fp32r), exponentiated on the ScalarEngine straight out of PSUM,
and contracted against [V | 1] without the N x N matrix ever leaving the
chip.  The trailing all-ones column of Vp1 yields the softmax denominator
as row 64 of the same accumulation.

Sharding: core c -> batch b = c//2, heads h0 = 4*(c%2) .. h0+3.
"""

import numpy as np

import concourse.bacc as bacc
import concourse.tile as tile
import concourse.mybir as mybir
from concourse.bass_utils import run_bass_kernel_spmd

B, N, IN, F, H = 4, 2048, 256, 64, 8
NH = H // 2          # heads per core
NT = N // 128        # 16 i-tiles / j-chunks
F32 = mybir.dt.float32
F32R = mybir.dt.float32r
AF = mybir.ActivationFunctionType
ALU = mybir.AluOpType

N_CORES = 8
_CACHE = {}


def build_nc():
    nc = bacc.Bacc("TRN2", target_bir_lowering=False, debug=False,
                   num_devices=N_CORES)
    X_d = nc.dram_tensor("X", [N, IN], F32, kind="ExternalInput")
    vW_d = nc.dram_tensor("vW", [IN, F], F32, kind="ExternalInput")
    vb_d = nc.dram_tensor("vb", [F], F32, kind="ExternalInput")
    qw_d = nc.dram_tensor("qw", [F, NH], F32, kind="ExternalInput")
    kw_d = nc.dram_tensor("kw", [F, NH], F32, kind="ExternalInput")
    qb_d = nc.dram_tensor("qb", [NH], F32, kind="ExternalInput")
    kb_d = nc.dram_tensor("kb", [NH], F32, kind="ExternalInput")
    id_d = nc.dram_tensor("ident", [128, 128], F32, kind="ExternalInput")
    out_d = nc.dram_tensor("out", [N, NH * F], F32, kind="ExternalOutput")

    with tile.TileContext(nc) as tc:
        with tc.tile_pool(name="persist", bufs=1) as pp:
            ident = pp.tile([128, 128], F32)
            nc.sync.dma_start(ident[:], id_d[:])
            id_r = pp.tile([128, 128], F32R)
            nc.vector.tensor_copy(id_r[:], ident[:])
            vt_sb = pp.tile([F, N], F32R)         # V^T, bias folded
            qt = pp.tile([NH, N], F32)
            kt = pp.tile([NH, N], F32)
            ab_hs = [pp.tile([2, N], F32R, name=f"abh{h}", tag=f"ab{h}") for h in range(NH)]
            pm_hs = [pp.tile([2, N], F32R, name=f"pmh{h}", tag=f"pm{h}") for h in range(NH)]
            vp1 = pp.tile([128, NT * (F + 1)], F32R)   # [V | 1] per j-tile

            # ---------- preamble: X^T, V^T, q/k ----------
            with tc.tile_pool(name="pre_sb", bufs=1) as sp:
                xsb = sp.tile([128, NT * IN], F32)
                nc.sync.dma_start(
                    xsb[:].rearrange("p (t c) -> p t c", t=NT),
                    X_d[:].rearrange("(t p) c -> p t c", p=128))
                vwsb = sp.tile([128, 128], F32)
                nc.sync.dma_start(
                    vwsb[:].rearrange("p (t f) -> p t f", t=2),
                    vW_d[:].rearrange("(t p) f -> p t f", p=128))
                vb_t = sp.tile([F, 1], F32)
                nc.sync.dma_start(vb_t[:], vb_d[:].unsqueeze(1))
                qw_t = sp.tile([F, NH], F32)
                nc.sync.dma_start(qw_t[:], qw_d[:])
                kw_t = sp.tile([F, NH], F32)
                nc.sync.dma_start(kw_t[:], kw_d[:])
                qb_t = sp.tile([NH, 1], F32)
                nc.sync.dma_start(qb_t[:], qb_d[:].unsqueeze(1))
                kb_t = sp.tile([NH, 1], F32)
                nc.sync.dma_start(kb_t[:], kb_d[:].unsqueeze(1))

                xt = sp.tile([128, 2 * N], F32R)  # X^T: chunk cc at cc*N
                vw_r = sp.tile([128, 128], F32R)
                nc.vector.tensor_copy(vw_r[:], vwsb[:])
                qw_r = sp.tile([F, NH], F32R)
                nc.vector.tensor_copy(qw_r[:], qw_t[:])
                kw_r = sp.tile([F, NH], F32R)
                nc.vector.tensor_copy(kw_r[:], kw_t[:])

                with tc.tile_pool(name="pre_ps", bufs=2, space="PSUM") as xp:
                    for t in range(NT):
                        for cc in range(2):
                            tp = xp.tile([128, 128], F32)
                            nc.tensor.transpose(
                                tp[:], xsb[:, t * IN + cc * 128:
                                           t * IN + cc * 128 + 128], ident[:])
                            nc.vector.tensor_copy(
                                xt[:, cc * N + t * 128: cc * N + t * 128 + 128],
                                tp[:])

                with tc.tile_pool(name="vt_ps", bufs=1, space="PSUM") as vpp:
                    vt_ps = vpp.tile([F, N], F32)
                    for nb in range(4):
                        for cc in range(2):
                            nc.tensor.matmul(
                                vt_ps[:, nb * 512: nb * 512 + 512],
                                vw_r[:, cc * F: cc * F + F],
                                xt[:, cc * N + nb * 512: cc * N + nb * 512 + 512],
                                start=(cc == 0), stop=(cc == 1))
                    nc.vector.tensor_scalar_add(vt_sb[:], vt_ps[:], vb_t[:])

                with tc.tile_pool(name="qk_ps", bufs=1, space="PSUM") as qpp:
                    qt_ps = qpp.tile([NH, N], F32)
                    kt_ps = qpp.tile([NH, N], F32)
                    for nb in range(4):
                        nc.tensor.matmul(
                            qt_ps[:, nb * 512: nb * 512 + 512], qw_r[:],
                            vt_sb[:, nb * 512: nb * 512 + 512],
                            start=True, stop=True)
                        nc.tensor.matmul(
                            kt_ps[:, nb * 512: nb * 512 + 512], kw_r[:],
                            vt_sb[:, nb * 512: nb * 512 + 512],
                            start=True, stop=True)
                    nc.vector.tensor_scalar_add(qt[:], qt_ps[:], qb_t[:])
                    nc.vector.tensor_scalar_add(kt[:], kt_ps[:], kb_t[:])

            # ---------- per-head vectors (fp32r) ----------
            with tc.tile_pool(name="vec_sb", bufs=1) as vs:
                a4 = vs.tile([NH, N], F32R)
                b4 = vs.tile([NH, N], F32R)
                p4 = vs.tile([NH, N], F32R)
                m4 = vs.tile([NH, N], F32R)
                nc.vector.scalar_tensor_tensor(a4[:], kt[:], 0.01, kt[:],
                                               ALU.mult, ALU.max)
                nc.vector.scalar_tensor_tensor(b4[:], kt[:], 0.01, kt[:],
                                               ALU.mult, ALU.min)
                nc.vector.tensor_scalar_max(p4[:], qt[:], 0.0)
                nc.vector.tensor_scalar_min(m4[:], qt[:], 0.0)
                for h in range(NH):
                    nc.sync.dma_start(ab_hs[h][0:1, :], a4[h:h + 1, :])
                    nc.sync.dma_start(ab_hs[h][1:2, :], b4[h:h + 1, :])
                    nc.sync.dma_start(pm_hs[h][0:1, :], p4[h:h + 1, :])
                    nc.sync.dma_start(pm_hs[h][1:2, :], m4[h:h + 1, :])

            # ---------- Vp1 = [V | 1] per j-tile ----------
            nc.vector.memset(vp1[:].bitcast(F32), 1.0)
            with tc.tile_pool(name="v_ps", bufs=2, space="PSUM") as vp:
                for t in range(NT):
                    v_ps = vp.tile([128, F], F32R)
                    nc.tensor.transpose(
                        v_ps[:], vt_sb[:, t * 128: t * 128 + 128],
                        id_r[0:F, 0:F])
                    nc.vector.tensor_copy(
                        vp1[:, t * (F + 1): t * (F + 1) + F], v_ps[:])

            # ---------- main loop ----------
            hsbs = {}
            with tc.tile_pool(name="lt_ps", bufs=3, space="PSUM") as ltp, \
                 tc.tile_pool(name="acc_ps", bufs=1, space="PSUM") as accp, \
                 tc.tile_pool(name="et_sb", bufs=3) as etp:
                for h in range(NH):
                    ab_h = ab_hs[h][:]
                    pm_h = pm_hs[h][:]
                    for ib in range(2):
                        acc = accp.tile([F + 1, 1024], F32, tag="acc")
                        for jc in range(NT):
                            lt = ltp.tile([128, 1024], F32, tag="lt")
                            for hf in range(2):
                                nc.tensor.matmul(
                                    lt[:, hf * 512: hf * 512 + 512],
                                    ab_h[:, jc * 128: jc * 128 + 128],
                                    pm_h[:, ib * 1024 + hf * 512:
                                         ib * 1024 + hf * 512 + 512],
                                    start=True, stop=True)
                            et = etp.tile([128, 1024], F32R, tag="et")
                            nc.scalar.activation(et[:], lt[:], AF.Exp)
                            for hf in range(2):
                                nc.tensor.matmul(
                                    acc[:, hf * 512: hf * 512 + 512],
                                    vp1[:, jc * (F + 1): (jc + 1) * (F + 1)],
                                    et[:, hf * 512: hf * 512 + 512],
                                    start=(jc == 0), stop=(jc == NT - 1))
                        hsb = pp.tile([F + 1, 1024], F32, name=f"hsb{h}_{ib}",
                                      tag=f"hsb{h}_{ib}")
                        nc.vector.tensor_copy(hsb[:], acc[:])
                        hsbs[(h, ib)] = hsb

            # ---------- postamble: transpose + normalize + store ----------
            with tc.tile_pool(name="ht_ps", bufs=4, space="PSUM") as htp, \
                 tc.tile_pool(name="post_sb", bufs=4) as postp:
                for h in range(NH):
                    for ib in range(2):
                        hsb = hsbs[(h, ib)]
                        for t8 in range(8):
                            ht = htp.tile([128, F + 1], F32, tag="ht")
                            nc.tensor.transpose(
                                ht[:], hsb[:, t8 * 128: t8 * 128 + 128],
                                ident[0:F + 1, 0:F + 1])
                            rcp = postp.tile([128, 1], F32, tag="rcp")
                            nc.vector.reciprocal(rcp[:], ht[:, F:F + 1])
                            ob = postp.tile([128, F], F32, tag="ob")
                            nc.vector.tensor_scalar_mul(ob[:], ht[:, 0:F], rcp[:])
                            r0 = ib * 1024 + t8 * 128
                            nc.sync.dma_start(
                                out_d[r0:r0 + 128, h * F: h * F + F], ob[:])
    nc.compile()
    return nc


def _get_nc():
    if "nc" not in _CACHE:
        _CACHE["nc"] = build_nc()
    return _CACHE["nc"]


def make_in_maps(X, vW, vb, qW, qb, kW, kb):
    ident = np.eye(128, dtype=np.float32)
    in_maps = []
    for c in range(N_CORES):
        b, h0 = c // 2, NH * (c % 2)
        in_maps.append({
            "X": np.ascontiguousarray(X[b]),
            "vW": np.ascontiguousarray(vW),
            "vb": np.ascontiguousarray(vb),
            "qw": np.ascontiguousarray(qW[:, h0:h0 + NH]),
            "kw": np.ascontiguousarray(kW[:, h0:h0 + NH]),
            "qb": np.ascontiguousarray(qb[h0:h0 + NH]),
            "kb": np.ascontiguousarray(kb[h0:h0 + NH]),
            "ident": ident,
        })
    return in_maps


def assemble(results):
    full = np.empty((B, N, H * F), dtype=np.float32)
    for c in range(N_CORES):
        b, h0 = c // 2, NH * (c % 2)
        full[b][:, h0 * F:(h0 + NH) * F] = results[c]["out"]
    return full


def kernel(X, vW, vb, qW, qb, kW, kb):
    X, vW, vb = np.asarray(X), np.asarray(vW), np.asarray(vb)
    qW, qb, kW, kb = np.asarray(qW), np.asarray(qb), np.asarray(kW), np.asarray(kb)
    nc = _get_nc()
    res = run_bass_kernel_spmd(nc, make_in_maps(X, vW, vb, qW, qb, kW, kb),
                               list(range(N_CORES)))
    return assemble(res.results)



# revision 3
# speedup vs baseline: 400.8246x; 400.8246x over previous
"""Multi-head graph attention (rank-2 LeakyReLU-softmax) Trainium2 kernel.

Reference computation (per batch b, head h):
    V = X @ vW + vb                       (N, F)
    q = V @ qW[:,h] + qb[h]               (N,)   per-node scalar
    k = V @ kW[:,h] + kb[h]               (N,)
    A_ij = softmax_j( LeakyReLU(q_i * k_j) )
    out[b,i,h,:] = sum_j A_ij V_j

Key identity used here: with P = max(q,0), M = min(q,0),
alpha = LeakyReLU(k) = max(k, 0.01k), beta = min(k, 0.01k),
    LeakyReLU(q_i * k_j) == alpha_j * P_i + beta_j * M_i      (exactly)
since for each i exactly one of P_i / M_i is nonzero.  So the N x N logit
matrix is a rank-2 outer product, built on the TensorEngine as a K=2
matmul (fp32r), exponentiated on the ScalarEngine straight out of PSUM,
and contracted against [V | 1] without the N x N matrix ever leaving the
chip.  The trailing all-ones column of Vp1 yields the softmax denominator
as row 64 of the same accumulation.

Sharding: core c -> batch b = c//2, heads h0 = 4*(c%2) .. h0+3.
"""

import numpy as np

import concourse.bacc as bacc
import concourse.tile as tile
import concourse.mybir as mybir
from concourse.bass_utils import run_bass_kernel_spmd

B, N, IN, F, H = 4, 2048, 256, 64, 8
NH = H // 2          # heads per core
NT = N // 128        # 16 i-tiles / j-chunks
F32 = mybir.dt.float32
F32R = mybir.dt.float32r
AF = mybir.ActivationFunctionType
ALU = mybir.AluOpType

N_CORES = 8
_CACHE = {}


def build_nc(reps=1):
    """Build the kernel program.

    reps > 1 wraps the whole computation in a hardware For_i loop (all-engine
    barrier between iterations) so test.py can measure per-execution HW time
    by slope: (t(R) - t(1)) / (R - 1).  The graded kernel() path uses reps=1.
    """
    nc = bacc.Bacc("TRN2", target_bir_lowering=False, debug=False,
                   num_devices=N_CORES)
    X_d = nc.dram_tensor("X", [N, IN], F32, kind="ExternalInput")
    vW_d = nc.dram_tensor("vW", [IN, F], F32, kind="ExternalInput")
    vb_d = nc.dram_tensor("vb", [F], F32, kind="ExternalInput")
    qw_d = nc.dram_tensor("qw", [F, NH], F32, kind="ExternalInput")
    kw_d = nc.dram_tensor("kw", [F, NH], F32, kind="ExternalInput")
    qb_d = nc.dram_tensor("qb", [NH], F32, kind="ExternalInput")
    kb_d = nc.dram_tensor("kb", [NH], F32, kind="ExternalInput")
    id_d = nc.dram_tensor("ident", [128, 128], F32, kind="ExternalInput")
    out_d = nc.dram_tensor("out", [N, NH * F], F32, kind="ExternalOutput")

    with tile.TileContext(nc) as tc:
        from contextlib import ExitStack
        with ExitStack() as rep_ctx:
            if reps > 1:
                rep_ctx.enter_context(tc.For_i(0, reps))
            _emit_body(nc, tc, X_d, vW_d, vb_d, qw_d, kw_d, qb_d, kb_d,
                       id_d, out_d)
    nc.compile()
    return nc


def _emit_body(nc, tc, X_d, vW_d, vb_d, qw_d, kw_d, qb_d, kb_d, id_d, out_d):
    if True:
        with tc.tile_pool(name="persist", bufs=1) as pp:
            ident = pp.tile([128, 128], F32)
            nc.sync.dma_start(ident[:], id_d[:])
            id_r = pp.tile([128, 128], F32R)
            nc.vector.tensor_copy(id_r[:], ident[:])
            vt_sb = pp.tile([F, N], F32R)         # V^T, bias folded
            qt = pp.tile([NH, N], F32)
            kt = pp.tile([NH, N], F32)
            ab_hs = [pp.tile([2, N], F32R, name=f"abh{h}", tag=f"ab{h}") for h in range(NH)]
            pm_hs = [pp.tile([2, N], F32R, name=f"pmh{h}", tag=f"pm{h}") for h in range(NH)]
            vp1 = pp.tile([128, NT * (F + 1)], F32R)   # [V | 1] per j-tile

            # ---------- preamble: X^T, V^T, q/k ----------
            with tc.tile_pool(name="pre_sb", bufs=1) as sp:
                xsb = sp.tile([128, NT * IN], F32)
                nc.sync.dma_start(
                    xsb[:].rearrange("p (t c) -> p t c", t=NT),
                    X_d[:].rearrange("(t p) c -> p t c", p=128))
                vwsb = sp.tile([128, 128], F32)
                nc.sync.dma_start(
                    vwsb[:].rearrange("p (t f) -> p t f", t=2),
                    vW_d[:].rearrange("(t p) f -> p t f", p=128))
                vb_t = sp.tile([F, 1], F32)
                nc.sync.dma_start(vb_t[:], vb_d[:].unsqueeze(1))
                qw_t = sp.tile([F, NH], F32)
                nc.sync.dma_start(qw_t[:], qw_d[:])
                kw_t = sp.tile([F, NH], F32)
                nc.sync.dma_start(kw_t[:], kw_d[:])
                qb_t = sp.tile([NH, 1], F32)
                nc.sync.dma_start(qb_t[:], qb_d[:].unsqueeze(1))
                kb_t = sp.tile([NH, 1], F32)
                nc.sync.dma_start(kb_t[:], kb_d[:].unsqueeze(1))

                xt = sp.tile([128, 2 * N], F32R)  # X^T: chunk cc at cc*N
                vw_r = sp.tile([128, 128], F32R)
                nc.vector.tensor_copy(vw_r[:], vwsb[:])
                qw_r = sp.tile([F, NH], F32R)
                nc.vector.tensor_copy(qw_r[:], qw_t[:])
                kw_r = sp.tile([F, NH], F32R)
                nc.vector.tensor_copy(kw_r[:], kw_t[:])

                with tc.tile_pool(name="pre_ps", bufs=2, space="PSUM") as xp:
                    for t in range(NT):
                        for cc in range(2):
                            tp = xp.tile([128, 128], F32)
                            nc.tensor.transpose(
                                tp[:], xsb[:, t * IN + cc * 128:
                                           t * IN + cc * 128 + 128], ident[:])
                            nc.vector.tensor_copy(
                                xt[:, cc * N + t * 128: cc * N + t * 128 + 128],
                                tp[:])

                with tc.tile_pool(name="vt_ps", bufs=1, space="PSUM") as vpp:
                    vt_ps = vpp.tile([F, N], F32)
                    for nb in range(4):
                        for cc in range(2):
                            nc.tensor.matmul(
                                vt_ps[:, nb * 512: nb * 512 + 512],
                                vw_r[:, cc * F: cc * F + F],
                                xt[:, cc * N + nb * 512: cc * N + nb * 512 + 512],
                                start=(cc == 0), stop=(cc == 1))
                    nc.vector.tensor_scalar_add(vt_sb[:], vt_ps[:], vb_t[:])

                with tc.tile_pool(name="qk_ps", bufs=1, space="PSUM") as qpp:
                    qt_ps = qpp.tile([NH, N], F32)
                    kt_ps = qpp.tile([NH, N], F32)
                    for nb in range(4):
                        nc.tensor.matmul(
                            qt_ps[:, nb * 512: nb * 512 + 512], qw_r[:],
                            vt_sb[:, nb * 512: nb * 512 + 512],
                            start=True, stop=True)
                        nc.tensor.matmul(
                            kt_ps[:, nb * 512: nb * 512 + 512], kw_r[:],
                            vt_sb[:, nb * 512: nb * 512 + 512],
                            start=True, stop=True)
                    nc.vector.tensor_scalar_add(qt[:], qt_ps[:], qb_t[:])
                    nc.vector.tensor_scalar_add(kt[:], kt_ps[:], kb_t[:])

            # ---------- per-head vectors (fp32r) ----------
            with tc.tile_pool(name="vec_sb", bufs=1) as vs:
                a4 = vs.tile([NH, N], F32R)
                b4 = vs.tile([NH, N], F32R)
                p4 = vs.tile([NH, N], F32R)
                m4 = vs.tile([NH, N], F32R)
                nc.vector.scalar_tensor_tensor(a4[:], kt[:], 0.01, kt[:],
                                               ALU.mult, ALU.max)
                nc.vector.scalar_tensor_tensor(b4[:], kt[:], 0.01, kt[:],
                                               ALU.mult, ALU.min)
                nc.vector.tensor_scalar_max(p4[:], qt[:], 0.0)
                nc.vector.tensor_scalar_min(m4[:], qt[:], 0.0)
                for h in range(NH):
                    nc.sync.dma_start(ab_hs[h][0:1, :], a4[h:h + 1, :])
                    nc.sync.dma_start(ab_hs[h][1:2, :], b4[h:h + 1, :])
                    nc.sync.dma_start(pm_hs[h][0:1, :], p4[h:h + 1, :])
                    nc.sync.dma_start(pm_hs[h][1:2, :], m4[h:h + 1, :])

            # ---------- Vp1 = [V | 1] per j-tile ----------
            nc.vector.memset(vp1[:].bitcast(F32), 1.0)
            with tc.tile_pool(name="v_ps", bufs=2, space="PSUM") as vp:
                for t in range(NT):
                    v_ps = vp.tile([128, F], F32R)
                    nc.tensor.transpose(
                        v_ps[:], vt_sb[:, t * 128: t * 128 + 128],
                        id_r[0:F, 0:F])
                    nc.vector.tensor_copy(
                        vp1[:, t * (F + 1): t * (F + 1) + F], v_ps[:])

            # ---------- main loop ----------
            hsbs = {}
            with tc.tile_pool(name="lt_ps", bufs=3, space="PSUM") as ltp, \
                 tc.tile_pool(name="acc_ps", bufs=1, space="PSUM") as accp, \
                 tc.tile_pool(name="et_sb", bufs=3) as etp:
                for h in range(NH):
                    ab_h = ab_hs[h][:]
                    pm_h = pm_hs[h][:]
                    for ib in range(2):
                        acc = accp.tile([F + 1, 1024], F32, tag="acc")
                        for jc in range(NT):
                            lt = ltp.tile([128, 1024], F32, tag="lt")
                            for hf in range(2):
                                nc.tensor.matmul(
                                    lt[:, hf * 512: hf * 512 + 512],
                                    ab_h[:, jc * 128: jc * 128 + 128],
                                    pm_h[:, ib * 1024 + hf * 512:
                                         ib * 1024 + hf * 512 + 512],
                                    start=True, stop=True)
                            et = etp.tile([128, 1024], F32R, tag="et")
                            nc.scalar.activation(et[:], lt[:], AF.Exp)
                            for hf in range(2):
                                nc.tensor.matmul(
                                    acc[:, hf * 512: hf * 512 + 512],
                                    vp1[:, jc * (F + 1): (jc + 1) * (F + 1)],
                                    et[:, hf * 512: hf * 512 + 512],
                                    start=(jc == 0), stop=(jc == NT - 1))
                        hsb = pp.tile([F + 1, 1024], F32, name=f"hsb{h}_{ib}",
                                      tag=f"hsb{h}_{ib}")
                        nc.vector.tensor_copy(hsb[:], acc[:])
                        hsbs[(h, ib)] = hsb

            # ---------- postamble: transpose + normalize + store ----------
            with tc.tile_pool(name="ht_ps", bufs=4, space="PSUM") as htp, \
                 tc.tile_pool(name="post_sb", bufs=4) as postp:
                for h in range(NH):
                    for ib in range(2):
                        hsb = hsbs[(h, ib)]
                        for t8 in range(8):
                            ht = htp.tile([128, F + 1], F32, tag="ht")
                            nc.tensor.transpose(
                                ht[:], hsb[:, t8 * 128: t8 * 128 + 128],
                                ident[0:F + 1, 0:F + 1])
                            rcp = postp.tile([128, 1], F32, tag="rcp")
                            nc.vector.reciprocal(rcp[:], ht[:, F:F + 1])
                            ob = postp.tile([128, F], F32, tag="ob")
                            nc.vector.tensor_scalar_mul(ob[:], ht[:, 0:F], rcp[:])
                            r0 = ib * 1024 + t8 * 128
                            nc.sync.dma_start(
                                out_d[r0:r0 + 128, h * F: h * F + F], ob[:])


def _get_nc():
    if "nc" not in _CACHE:
        _CACHE["nc"] = build_nc()
    return _CACHE["nc"]


def make_in_maps(X, vW, vb, qW, qb, kW, kb):
    ident = np.eye(128, dtype=np.float32)
    in_maps = []
    for c in range(N_CORES):
        b, h0 = c // 2, NH * (c % 2)
        in_maps.append({
            "X": np.ascontiguousarray(X[b]),
            "vW": np.ascontiguousarray(vW),
            "vb": np.ascontiguousarray(vb),
            "qw": np.ascontiguousarray(qW[:, h0:h0 + NH]),
            "kw": np.ascontiguousarray(kW[:, h0:h0 + NH]),
            "qb": np.ascontiguousarray(qb[h0:h0 + NH]),
            "kb": np.ascontiguousarray(kb[h0:h0 + NH]),
            "ident": ident,
        })
    return in_maps


def assemble(results):
    full = np.empty((B, N, H * F), dtype=np.float32)
    for c in range(N_CORES):
        b, h0 = c // 2, NH * (c % 2)
        full[b][:, h0 * F:(h0 + NH) * F] = results[c]["out"]
    return full


def kernel(X, vW, vb, qW, qb, kW, kb):
    X, vW, vb = np.asarray(X), np.asarray(vW), np.asarray(vb)
    qW, qb, kW, kb = np.asarray(qW), np.asarray(qb), np.asarray(kW), np.asarray(kb)
    nc = _get_nc()
    res = run_bass_kernel_spmd(nc, make_in_maps(X, vW, vb, qW, qb, kW, kb),
                               list(range(N_CORES)))
    return assemble(res.results)



# revision 6
# speedup vs baseline: 1305.7669x; 3.2577x over previous
"""Multi-head graph attention (rank-2 LeakyReLU-softmax) Trainium2 kernel.

Reference computation (per batch b, head h):
    V = X @ vW + vb                       (N, F)
    q = V @ qW[:,h] + qb[h]               (N,)   per-node scalar
    k = V @ kW[:,h] + kb[h]               (N,)
    A_ij = softmax_j( LeakyReLU(q_i * k_j) )
    out[b,i,h,:] = sum_j A_ij V_j

Key identity used here: with P = max(q,0), M = min(q,0),
alpha = LeakyReLU(k) = max(k, 0.01k), beta = min(k, 0.01k),
    LeakyReLU(q_i * k_j) == alpha_j * P_i + beta_j * M_i      (exactly)
since for each i exactly one of P_i / M_i is nonzero.  So the N x N logit
matrix is a rank-2 outer product, built on the TensorEngine as a K=2
matmul (fp32r), exponentiated on the ScalarEngine straight out of PSUM,
and contracted against [V | 1] without the N x N matrix ever leaving the
chip.  The trailing all-ones column of Vp1 yields the softmax denominator
as row 64 of the same accumulation.

Sharding: core c -> batch b = c//2, heads h0 = 4*(c%2) .. h0+3.
"""

import numpy as np

import concourse.bacc as bacc
import concourse.tile as tile
import concourse.mybir as mybir
from concourse.bass_utils import run_bass_kernel_spmd

B, N, IN, F, H = 4, 2048, 256, 64, 8
NH = H // 2          # heads per core
NT = N // 128        # 16 i-tiles / j-chunks
F32 = mybir.dt.float32
F32R = mybir.dt.float32r
AF = mybir.ActivationFunctionType
ALU = mybir.AluOpType

N_CORES = 8
_CACHE = {}


def build_nc(reps=1, unroll=False, version=2):
    """Build the kernel program.

    reps > 1 wraps the whole computation in a hardware For_i loop (all-engine
    barrier between iterations) so test.py can measure per-execution HW time
    by slope: (t(R) - t(1)) / (R - 1).  The graded kernel() path uses reps=1.
    """
    nc = bacc.Bacc("TRN2", target_bir_lowering=False, debug=False,
                   num_devices=N_CORES)
    X_d = nc.dram_tensor("X", [N, IN], F32, kind="ExternalInput")
    vW_d = nc.dram_tensor("vW", [IN, F], F32, kind="ExternalInput")
    vb_d = nc.dram_tensor("vb", [F], F32, kind="ExternalInput")
    qw_d = nc.dram_tensor("qw", [F, NH], F32, kind="ExternalInput")
    kw_d = nc.dram_tensor("kw", [F, NH], F32, kind="ExternalInput")
    qb_d = nc.dram_tensor("qb", [NH], F32, kind="ExternalInput")
    kb_d = nc.dram_tensor("kb", [NH], F32, kind="ExternalInput")
    id_d = nc.dram_tensor("ident", [128, 128], F32, kind="ExternalInput")
    out_d = nc.dram_tensor("out", [N, NH * F], F32, kind="ExternalOutput")

    body = _emit_body if version == 1 else _emit_body_v2
    with tile.TileContext(nc) as tc:
        from contextlib import ExitStack
        with ExitStack() as rep_ctx:
            if reps > 1 and not unroll:
                rep_ctx.enter_context(tc.For_i(0, reps))
            for _ in range(reps if unroll else 1):
                body(nc, tc, X_d, vW_d, vb_d, qw_d, kw_d, qb_d, kb_d,
                     id_d, out_d)
    nc.compile()
    return nc


def _emit_body_v2(nc, tc, X_d, vW_d, vb_d, qw_d, kw_d, qb_d, kb_d, id_d,
                  out_d):
    """Software-pipelined main loop.

    Per (head, i-block) "block" (NB = NH*2 of them), per j-chunk step:
      PE:  logit matmul (K=2 rank-2 outer product) -> lt PSUM [128,1024]
      Act: exp straight out of PSUM -> et SBUF (the ONLY Act work)
      PE:  acc matmul [V|1]^T @ et -> acc PSUM [65,1024] accumulated over 16 j
    Steps are emitted with a 1-step skew (logit(s) before acc(s-1)) so PE's
    in-order queue never parks an exp-dependent acc in front of independent
    logit work.  Postamble (PE transpose + DVE normalize into an SBUF staging
    buffer) is interleaved into the following block's steps; output leaves the
    chip in one final DMA.
    """
    NB = NH * 2
    blocks = [(h, ib) for h in range(NH) for ib in range(2)]
    with tc.tile_pool(name="persist", bufs=1) as pp:
        ident = pp.tile([128, 128], F32)
        nc.sync.dma_start(ident[:], id_d[:])
        id_r = pp.tile([128, 128], F32R)
        nc.vector.tensor_copy(id_r[:], ident[:])
        vt_sb = pp.tile([F, N], F32R)         # V^T, bias folded
        qt = pp.tile([NH, N], F32)
        kt = pp.tile([NH, N], F32)
        ab8 = pp.tile([2 * NH, N], F32R)      # rows 2h/2h+1 = alpha_h/beta_h
        pm8 = pp.tile([2 * NH, N], F32R)      # rows 2h/2h+1 = P_h/M_h
        vp1 = pp.tile([128, NT * (F + 1)], F32R)   # [V | 1] per j-tile
        obuf = pp.tile([128, NT * NH * F], F32)    # staged output

        # ---------- preamble: X^T, V^T, q/k ----------
        with tc.tile_pool(name="pre_sb", bufs=1) as sp:
            xsb = sp.tile([128, NT * IN], F32)
            nc.sync.dma_start(
                xsb[:].rearrange("p (t c) -> p t c", t=NT),
                X_d[:].rearrange("(t p) c -> p t c", p=128))
            vwsb = sp.tile([128, 128], F32)
            nc.sync.dma_start(
                vwsb[:].rearrange("p (t f) -> p t f", t=2),
                vW_d[:].rearrange("(t p) f -> p t f", p=128))
            vb_t = sp.tile([F, 1], F32)
            nc.sync.dma_start(vb_t[:], vb_d[:].unsqueeze(1))
            qw_t = sp.tile([F, NH], F32)
            nc.sync.dma_start(qw_t[:], qw_d[:])
            kw_t = sp.tile([F, NH], F32)
            nc.sync.dma_start(kw_t[:], kw_d[:])
            qb_t = sp.tile([NH, 1], F32)
            nc.sync.dma_start(qb_t[:], qb_d[:].unsqueeze(1))
            kb_t = sp.tile([NH, 1], F32)
            nc.sync.dma_start(kb_t[:], kb_d[:].unsqueeze(1))

            xt = sp.tile([128, 2 * N], F32R)  # X^T: chunk cc at cc*N
            vw_r = sp.tile([128, 128], F32R)
            nc.vector.tensor_copy(vw_r[:], vwsb[:])
            qw_r = sp.tile([F, NH], F32R)
            nc.vector.tensor_copy(qw_r[:], qw_t[:])
            kw_r = sp.tile([F, NH], F32R)
            nc.vector.tensor_copy(kw_r[:], kw_t[:])

            with tc.tile_pool(name="pre_ps", bufs=2, space="PSUM") as xp:
                for t in range(NT):
                    for cc in range(2):
                        tp = xp.tile([128, 128], F32)
                        nc.tensor.transpose(
                            tp[:], xsb[:, t * IN + cc * 128:
                                       t * IN + cc * 128 + 128], ident[:])
                        nc.vector.tensor_copy(
                            xt[:, cc * N + t * 128: cc * N + t * 128 + 128],
                            tp[:])

            with tc.tile_pool(name="vt_ps", bufs=1, space="PSUM") as vpp:
                vt_ps = vpp.tile([F, N], F32)
                for nb in range(4):
                    for cc in range(2):
                        nc.tensor.matmul(
                            vt_ps[:, nb * 512: nb * 512 + 512],
                            vw_r[:, cc * F: cc * F + F],
                            xt[:, cc * N + nb * 512: cc * N + nb * 512 + 512],
                            start=(cc == 0), stop=(cc == 1))
                nc.vector.tensor_scalar_add(vt_sb[:], vt_ps[:], vb_t[:])

            with tc.tile_pool(name="qk_ps", bufs=1, space="PSUM") as qpp:
                qt_ps = qpp.tile([NH, N], F32)
                kt_ps = qpp.tile([NH, N], F32)
                for nb in range(4):
                    nc.tensor.matmul(
                        qt_ps[:, nb * 512: nb * 512 + 512], qw_r[:],
                        vt_sb[:, nb * 512: nb * 512 + 512],
                        start=True, stop=True)
                    nc.tensor.matmul(
                        kt_ps[:, nb * 512: nb * 512 + 512], kw_r[:],
                        vt_sb[:, nb * 512: nb * 512 + 512],
                        start=True, stop=True)
                nc.vector.tensor_scalar_add(qt[:], qt_ps[:], qb_t[:])
                nc.vector.tensor_scalar_add(kt[:], kt_ps[:], kb_t[:])

            # per-head vectors, written [alpha0..3 | beta0..3] then paired
            abq = sp.tile([2 * NH, N], F32R)
            pmq = sp.tile([2 * NH, N], F32R)
            nc.vector.scalar_tensor_tensor(abq[0:NH, :], kt[:], 0.01, kt[:],
                                           ALU.mult, ALU.max)
            nc.vector.scalar_tensor_tensor(abq[NH:2 * NH, :], kt[:], 0.01,
                                           kt[:], ALU.mult, ALU.min)
            nc.vector.tensor_scalar_max(pmq[0:NH, :], qt[:], 0.0)
            nc.vector.tensor_scalar_min(pmq[NH:2 * NH, :], qt[:], 0.0)
            nc.sync.dma_start(
                ab8[:].rearrange("(h two) n -> h two n", two=2),
                abq[:].rearrange("(two h) n -> h two n", h=NH))
            nc.sync.dma_start(
                pm8[:].rearrange("(h two) n -> h two n", two=2),
                pmq[:].rearrange("(two h) n -> h two n", h=NH))

        # ---------- Vp1 = [V | 1] per j-tile ----------
        nc.vector.memset(vp1[:].bitcast(F32), 1.0)
        with tc.tile_pool(name="v_ps", bufs=2, space="PSUM") as vp:
            for t in range(NT):
                v_ps = vp.tile([128, F], F32R)
                nc.tensor.transpose(
                    v_ps[:], vt_sb[:, t * 128: t * 128 + 128],
                    id_r[0:F, 0:F])
                nc.vector.tensor_copy(
                    vp1[:, t * (F + 1): t * (F + 1) + F], v_ps[:])

        # ---------- software-pipelined main loop ----------
        S = NB * NT  # 128 steps
        with tc.tile_pool(name="lt_ps", bufs=2, space="PSUM") as ltp, \
             tc.tile_pool(name="acc_ps", bufs=1, space="PSUM") as accp, \
             tc.tile_pool(name="et_sb", bufs=4) as etp, \
             tc.tile_pool(name="post_sb", bufs=2) as postp:
            lts, ets, accs, hsbs = {}, {}, {}, {}

            def emit_logit(s):
                b, jc = divmod(s, NT)
                h, ib = blocks[b]
                lt = ltp.tile([128, 1024], F32, tag="lt")
                for hf in range(2):
                    nc.tensor.matmul(
                        lt[:, hf * 512: hf * 512 + 512],
                        ab8[2 * h: 2 * h + 2, jc * 128: jc * 128 + 128],
                        pm8[2 * h: 2 * h + 2,
                            ib * 1024 + hf * 512: ib * 1024 + hf * 512 + 512],
                        start=True, stop=True)
                lts[s] = lt

            def emit_exp(s):
                et = etp.tile([128, 1024], F32R, tag="et")
                nc.scalar.activation(et[:], lts[s][:], AF.Exp)
                ets[s] = et

            def emit_acc(s):
                b, jc = divmod(s, NT)
                if jc == 0:
                    accs[b] = accp.tile([F + 1, 1024], F32, tag="acc")
                for hf in range(2):
                    nc.tensor.matmul(
                        accs[b][:, hf * 512: hf * 512 + 512],
                        vp1[:, jc * (F + 1): (jc + 1) * (F + 1)],
                        ets[s][:, hf * 512: hf * 512 + 512],
                        start=(jc == 0), stop=(jc == NT - 1))

            def emit_hsb(b):
                hsb = postp.tile([F + 1, 1024], F32, tag="hsb")
                nc.vector.tensor_copy(hsb[:], accs[b][:])
                hsbs[b] = hsb

            def emit_trans(b, t8):
                h, ib = blocks[b]
                ht = ltp.tile([128, F + 1], F32, tag="ht")
                nc.tensor.transpose(
                    ht[:], hsbs[b][:, t8 * 128: t8 * 128 + 128],
                    ident[0:F + 1, 0:F + 1])
                rcp = postp.tile([128, 1], F32, tag="rcp")
                nc.vector.reciprocal(rcp[:], ht[:, F:F + 1])
                t = ib * 8 + t8
                nc.vector.tensor_scalar_mul(
                    obuf[:, t * (NH * F) + h * F: t * (NH * F) + h * F + F],
                    ht[:, 0:F], rcp[:])

            for s in range(S + 1):
                if s < S:
                    emit_logit(s)
                if s >= 1:
                    emit_exp(s - 1)
                    emit_acc(s - 1)
                    if (s - 1) % NT == NT - 1:
                        emit_hsb((s - 1) // NT)
                    # spread previous block's 8 transposes over this block
                    b_prev = s // NT - 1
                    jc = s % NT
                    if b_prev >= 0 and s < S and jc % 2 == 1:
                        emit_trans(b_prev, jc // 2)
            for t8 in range(8):  # drain last block
                emit_trans(NB - 1, t8)

        nc.sync.dma_start(
            out_d[:].rearrange("(t p) c -> p t c", p=128),
            obuf[:].rearrange("p (t c) -> p t c", t=NT))


def _emit_body(nc, tc, X_d, vW_d, vb_d, qw_d, kw_d, qb_d, kb_d, id_d, out_d):
    if True:
        with tc.tile_pool(name="persist", bufs=1) as pp:
            ident = pp.tile([128, 128], F32)
            nc.sync.dma_start(ident[:], id_d[:])
            id_r = pp.tile([128, 128], F32R)
            nc.vector.tensor_copy(id_r[:], ident[:])
            vt_sb = pp.tile([F, N], F32R)         # V^T, bias folded
            qt = pp.tile([NH, N], F32)
            kt = pp.tile([NH, N], F32)
            ab_hs = [pp.tile([2, N], F32R, name=f"abh{h}", tag=f"ab{h}") for h in range(NH)]
            pm_hs = [pp.tile([2, N], F32R, name=f"pmh{h}", tag=f"pm{h}") for h in range(NH)]
            vp1 = pp.tile([128, NT * (F + 1)], F32R)   # [V | 1] per j-tile

            # ---------- preamble: X^T, V^T, q/k ----------
            with tc.tile_pool(name="pre_sb", bufs=1) as sp:
                xsb = sp.tile([128, NT * IN], F32)
                nc.sync.dma_start(
                    xsb[:].rearrange("p (t c) -> p t c", t=NT),
                    X_d[:].rearrange("(t p) c -> p t c", p=128))
                vwsb = sp.tile([128, 128], F32)
                nc.sync.dma_start(
                    vwsb[:].rearrange("p (t f) -> p t f", t=2),
                    vW_d[:].rearrange("(t p) f -> p t f", p=128))
                vb_t = sp.tile([F, 1], F32)
                nc.sync.dma_start(vb_t[:], vb_d[:].unsqueeze(1))
                qw_t = sp.tile([F, NH], F32)
                nc.sync.dma_start(qw_t[:], qw_d[:])
                kw_t = sp.tile([F, NH], F32)
                nc.sync.dma_start(kw_t[:], kw_d[:])
                qb_t = sp.tile([NH, 1], F32)
                nc.sync.dma_start(qb_t[:], qb_d[:].unsqueeze(1))
                kb_t = sp.tile([NH, 1], F32)
                nc.sync.dma_start(kb_t[:], kb_d[:].unsqueeze(1))

                xt = sp.tile([128, 2 * N], F32R)  # X^T: chunk cc at cc*N
                vw_r = sp.tile([128, 128], F32R)
                nc.vector.tensor_copy(vw_r[:], vwsb[:])
                qw_r = sp.tile([F, NH], F32R)
                nc.vector.tensor_copy(qw_r[:], qw_t[:])
                kw_r = sp.tile([F, NH], F32R)
                nc.vector.tensor_copy(kw_r[:], kw_t[:])

                with tc.tile_pool(name="pre_ps", bufs=2, space="PSUM") as xp:
                    for t in range(NT):
                        for cc in range(2):
                            tp = xp.tile([128, 128], F32)
                            nc.tensor.transpose(
                                tp[:], xsb[:, t * IN + cc * 128:
                                           t * IN + cc * 128 + 128], ident[:])
                            nc.vector.tensor_copy(
                                xt[:, cc * N + t * 128: cc * N + t * 128 + 128],
                                tp[:])

                with tc.tile_pool(name="vt_ps", bufs=1, space="PSUM") as vpp:
                    vt_ps = vpp.tile([F, N], F32)
                    for nb in range(4):
                        for cc in range(2):
                            nc.tensor.matmul(
                                vt_ps[:, nb * 512: nb * 512 + 512],
                                vw_r[:, cc * F: cc * F + F],
                                xt[:, cc * N + nb * 512: cc * N + nb * 512 + 512],
                                start=(cc == 0), stop=(cc == 1))
                    nc.vector.tensor_scalar_add(vt_sb[:], vt_ps[:], vb_t[:])

                with tc.tile_pool(name="qk_ps", bufs=1, space="PSUM") as qpp:
                    qt_ps = qpp.tile([NH, N], F32)
                    kt_ps = qpp.tile([NH, N], F32)
                    for nb in range(4):
                        nc.tensor.matmul(
                            qt_ps[:, nb * 512: nb * 512 + 512], qw_r[:],
                            vt_sb[:, nb * 512: nb * 512 + 512],
                            start=True, stop=True)
                        nc.tensor.matmul(
                            kt_ps[:, nb * 512: nb * 512 + 512], kw_r[:],
                            vt_sb[:, nb * 512: nb * 512 + 512],
                            start=True, stop=True)
                    nc.vector.tensor_scalar_add(qt[:], qt_ps[:], qb_t[:])
                    nc.vector.tensor_scalar_add(kt[:], kt_ps[:], kb_t[:])

            # ---------- per-head vectors (fp32r) ----------
            with tc.tile_pool(name="vec_sb", bufs=1) as vs:
                a4 = vs.tile([NH, N], F32R)
                b4 = vs.tile([NH, N], F32R)
                p4 = vs.tile([NH, N], F32R)
                m4 = vs.tile([NH, N], F32R)
                nc.vector.scalar_tensor_tensor(a4[:], kt[:], 0.01, kt[:],
                                               ALU.mult, ALU.max)
                nc.vector.scalar_tensor_tensor(b4[:], kt[:], 0.01, kt[:],
                                               ALU.mult, ALU.min)
                nc.vector.tensor_scalar_max(p4[:], qt[:], 0.0)
                nc.vector.tensor_scalar_min(m4[:], qt[:], 0.0)
                for h in range(NH):
                    nc.sync.dma_start(ab_hs[h][0:1, :], a4[h:h + 1, :])
                    nc.sync.dma_start(ab_hs[h][1:2, :], b4[h:h + 1, :])
                    nc.sync.dma_start(pm_hs[h][0:1, :], p4[h:h + 1, :])
                    nc.sync.dma_start(pm_hs[h][1:2, :], m4[h:h + 1, :])

            # ---------- Vp1 = [V | 1] per j-tile ----------
            nc.vector.memset(vp1[:].bitcast(F32), 1.0)
            with tc.tile_pool(name="v_ps", bufs=2, space="PSUM") as vp:
                for t in range(NT):
                    v_ps = vp.tile([128, F], F32R)
                    nc.tensor.transpose(
                        v_ps[:], vt_sb[:, t * 128: t * 128 + 128],
                        id_r[0:F, 0:F])
                    nc.vector.tensor_copy(
                        vp1[:, t * (F + 1): t * (F + 1) + F], v_ps[:])

            # ---------- main loop ----------
            hsbs = {}
            with tc.tile_pool(name="lt_ps", bufs=3, space="PSUM") as ltp, \
                 tc.tile_pool(name="acc_ps", bufs=1, space="PSUM") as accp, \
                 tc.tile_pool(name="et_sb", bufs=3) as etp:
                for h in range(NH):
                    ab_h = ab_hs[h][:]
                    pm_h = pm_hs[h][:]
                    for ib in range(2):
                        acc = accp.tile([F + 1, 1024], F32, tag="acc")
                        for jc in range(NT):
                            lt = ltp.tile([128, 1024], F32, tag="lt")
                            for hf in range(2):
                                nc.tensor.matmul(
                                    lt[:, hf * 512: hf * 512 + 512],
                                    ab_h[:, jc * 128: jc * 128 + 128],
                                    pm_h[:, ib * 1024 + hf * 512:
                                         ib * 1024 + hf * 512 + 512],
                                    start=True, stop=True)
                            et = etp.tile([128, 1024], F32R, tag="et")
                            nc.scalar.activation(et[:], lt[:], AF.Exp)
                            for hf in range(2):
                                nc.tensor.matmul(
                                    acc[:, hf * 512: hf * 512 + 512],
                                    vp1[:, jc * (F + 1): (jc + 1) * (F + 1)],
                                    et[:, hf * 512: hf * 512 + 512],
                                    start=(jc == 0), stop=(jc == NT - 1))
                        hsb = pp.tile([F + 1, 1024], F32, name=f"hsb{h}_{ib}",
                                      tag=f"hsb{h}_{ib}")
                        nc.vector.tensor_copy(hsb[:], acc[:])
                        hsbs[(h, ib)] = hsb

            # ---------- postamble: transpose + normalize + store ----------
            with tc.tile_pool(name="ht_ps", bufs=4, space="PSUM") as htp, \
                 tc.tile_pool(name="post_sb", bufs=4) as postp:
                for h in range(NH):
                    for ib in range(2):
                        hsb = hsbs[(h, ib)]
                        for t8 in range(8):
                            ht = htp.tile([128, F + 1], F32, tag="ht")
                            nc.tensor.transpose(
                                ht[:], hsb[:, t8 * 128: t8 * 128 + 128],
                                ident[0:F + 1, 0:F + 1])
                            rcp = postp.tile([128, 1], F32, tag="rcp")
                            nc.vector.reciprocal(rcp[:], ht[:, F:F + 1])
                            ob = postp.tile([128, F], F32, tag="ob")
                            nc.vector.tensor_scalar_mul(ob[:], ht[:, 0:F], rcp[:])
                            r0 = ib * 1024 + t8 * 128
                            nc.sync.dma_start(
                                out_d[r0:r0 + 128, h * F: h * F + F], ob[:])


def _get_nc():
    if "nc" not in _CACHE:
        _CACHE["nc"] = build_nc()
    return _CACHE["nc"]


def make_in_maps(X, vW, vb, qW, qb, kW, kb):
    ident = np.eye(128, dtype=np.float32)
    in_maps = []
    for c in range(N_CORES):
        b, h0 = c // 2, NH * (c % 2)
        in_maps.append({
            "X": np.ascontiguousarray(X[b]),
            "vW": np.ascontiguousarray(vW),
            "vb": np.ascontiguousarray(vb),
            "qw": np.ascontiguousarray(qW[:, h0:h0 + NH]),
            "kw": np.ascontiguousarray(kW[:, h0:h0 + NH]),
            "qb": np.ascontiguousarray(qb[h0:h0 + NH]),
            "kb": np.ascontiguousarray(kb[h0:h0 + NH]),
            "ident": ident,
        })
    return in_maps


def assemble(results):
    full = np.empty((B, N, H * F), dtype=np.float32)
    for c in range(N_CORES):
        b, h0 = c // 2, NH * (c % 2)
        full[b][:, h0 * F:(h0 + NH) * F] = results[c]["out"]
    return full


def kernel(X, vW, vb, qW, qb, kW, kb):
    X, vW, vb = np.asarray(X), np.asarray(vW), np.asarray(vb)
    qW, qb, kW, kb = np.asarray(qW), np.asarray(qb), np.asarray(kW), np.asarray(kb)
    nc = _get_nc()
    res = run_bass_kernel_spmd(nc, make_in_maps(X, vW, vb, qW, qb, kW, kb),
                               list(range(N_CORES)))
    return assemble(res.results)

